# revision 6
# baseline (speedup 1.0000x reference)
"""AnyPrecisionLinear (4-bit LUT-quantized linear) Trainium2 kernel, 8-core SPMD.

y[b,s,o] = sum_i x[b,s,i] * lut[o, code[o,i]] + bias[o]
code assembled MSB-first from bitplanes 0..3 of qweight.

Sharding (column-parallel, per hint): out_features padded 11008->11264 and
split 1408 per core; x replicated; no collectives. Output gathered on host.

Per-core device pipeline:
  1. dequant per o-tile (128 rows): extract bit tiles with fused
     shift+is_lt tensor_scalar ops (DVE), 8 codebook "leaves"
     t_k = b3*d_k + lut[:,2k] on the Scalar engine (per-partition fp32
     scale/bias APs), then a 7-op copy_predicated binary tree (DVE) to
     select by bits b2,b1,b0.  Produces W[o-tile, i'] fp16.
  2. transpose each 128x128 slice on the TensorEngine (identity matmul)
     into WT[i', o] layout for the GEMM's stationary operand.
  3. GEMM: for each 256-token block, accumulate 32 K-tiles into PSUM;
     epilogue adds bias and casts to fp16 on the Scalar engine.

The in-feature axis is processed in a permuted order i' = jj*128 + w
(i = 32w + jj), which makes both bit extraction and the GEMM K-tiling
dense; the host applies the same permutation to x, which is
mathematically free (contraction is order-invariant).
"""

import numpy as np

IN = 4096
O_FULL = 11008
NCORES = 8
O_PAD = 11264          # 8 * 11 * 128
O_SH = O_PAD // NCORES  # 1408
OT = O_SH // 128        # 11 o-tiles
KT = IN // 128          # 32 k-tiles
T = 4096                # tokens
TBLK = 256
NTB = T // TBLK         # 16 token blocks
NQ = 8                  # bit-positions (jj) per dequant pass
NPASS = KT // NQ        # 4 passes per o-tile

_PROGRAM = None  # cached (nc, core_ids)


def _build_program():
    import concourse.mybir as mybir
    import concourse.tile as tile
    from concourse import bacc
    from concourse.masks import make_identity
    from contextlib import ExitStack

    nc = bacc.Bacc("TRN2", target_bir_lowering=False, debug=False,
                   num_devices=NCORES)

    qw_e = nc.dram_tensor("qw", [4, O_SH, 128], mybir.dt.int32,
                          kind="ExternalInput")
    lut_e = nc.dram_tensor("lut", [O_SH, 16], mybir.dt.float16,
                           kind="ExternalInput")
    bias_e = nc.dram_tensor("bias", [O_SH, 1], mybir.dt.float16,
                            kind="ExternalInput")
    xt_e = nc.dram_tensor("xt", [IN, T], mybir.dt.float16,
                          kind="ExternalInput")
    out_e = nc.dram_tensor("out", [O_SH, T], mybir.dt.float16,
                           kind="ExternalOutput")

    with tile.TileContext(nc) as tc:
        ctx = ExitStack()
        singles = ctx.enter_context(tc.tile_pool(name="singles", bufs=1))
        qpool = ctx.enter_context(tc.tile_pool(name="qpool", bufs=1))
        bpool = ctx.enter_context(tc.tile_pool(name="bpool", bufs=2))
        tpool = ctx.enter_context(tc.tile_pool(name="tpool", bufs=1))
        wpool = ctx.enter_context(tc.tile_pool(name="wpool", bufs=1))
        xpool = ctx.enter_context(tc.tile_pool(name="xpool", bufs=2))
        opool = ctx.enter_context(tc.tile_pool(name="opool", bufs=4))
        ps_tr = ctx.enter_context(tc.tile_pool(name="ps_tr", bufs=2,
                                               space="PSUM"))
        ps_mm = ctx.enter_context(tc.tile_pool(name="ps_mm", bufs=4,
                                               space="PSUM"))

        # --- constants -----------------------------------------------------
        ident = singles.tile([128, 128], mybir.dt.float16, name="ident")
        make_identity(nc, ident[:])

        lut_sb = singles.tile([128, OT, 16], mybir.dt.float16, name="lut_sb")
        nc.sync.dma_start(
            out=lut_sb[:],
            in_=lut_e.ap().rearrange("(ot p) c -> p ot c", p=128))
        lut32 = singles.tile([128, OT, 16], mybir.dt.float32, name="lut32")
        nc.vector.tensor_copy(out=lut32[:], in_=lut_sb[:])
        # d[:, ot, k] = lut[:, ot, 2k+1] - lut[:, ot, 2k]
        dq = singles.tile([128, OT, 8], mybir.dt.float32, name="dq")
        nc.vector.tensor_tensor(out=dq[:], in0=lut32[:, :, 1::2],
                                in1=lut32[:, :, 0::2],
                                op=mybir.AluOpType.subtract)

        bias_sb = singles.tile([128, OT], mybir.dt.float16, name="bias_sb")
        nc.sync.dma_start(
            out=bias_sb[:],
            in_=bias_e.ap().rearrange("(ot p) c -> p (ot c)", p=128))
        bias32 = singles.tile([128, OT], mybir.dt.float32, name="bias32")
        nc.vector.tensor_copy(out=bias32[:], in_=bias_sb[:])

        # persistent transposed weights: wt[ot][kt] is [128 i', 128 o] fp16
        wt = [[wpool.tile([128, 128], mybir.dt.float16,
                          name=f"wt_{ot}_{kt}", tag=f"wt_{ot}_{kt}")
               for kt in range(KT)] for ot in range(OT)]

        # --- dequant + transpose ------------------------------------------
        for ot in range(OT):
            qt = [qpool.tile([128, 128], mybir.dt.int32, name=f"qt{p}",
                             tag=f"qt{p}") for p in range(4)]
            for p in range(4):
                nc.sync.dma_start(out=qt[p][:], in_=qw_e[p, ot * 128:(ot + 1) * 128, :])

            for ps in range(NPASS):
                jj0 = ps * NQ
                # int32 0/1 bit tiles via fused bitwise shift+and (a bitwise
                # TensorScalar cannot cast, so extraction stays int32)
                bt = [bpool.tile([128, NQ * 128], mybir.dt.int32,
                                 name=f"bt{p}", tag=f"bt{p}")
                      for p in range(4)]
                for j in range(NQ):
                    for p in range(4):
                        nc.vector.tensor_scalar(
                            out=bt[p][:, j * 128:(j + 1) * 128],
                            in0=qt[p][:],
                            scalar1=31 - (jj0 + j),
                            scalar2=1,
                            op0=mybir.AluOpType.logical_shift_right,
                            op1=mybir.AluOpType.bitwise_and,
                        )
                # b3 as exact fp16 0/1 for the scalar-engine leaves
                b3f = bpool.tile([128, NQ * 128], mybir.dt.float16,
                                 name="b3f", tag="b3f")
                nc.vector.tensor_copy(out=b3f[:], in_=bt[3][:])
                tk = [tpool.tile([128, NQ * 128], mybir.dt.float16,
                                 name=f"tk{k}", tag=f"tk{k}")
                      for k in range(8)]
                for k in range(8):
                    nc.scalar.activation(
                        out=tk[k][:], in_=b3f[:],
                        func=mybir.ActivationFunctionType.Identity,
                        bias=lut32[:, ot, 2 * k:2 * k + 1],
                        scale=dq[:, ot, k:k + 1],
                    )
                for j in range(4):
                    nc.vector.copy_predicated(out=tk[2 * j][:], mask=bt[2][:],
                                              data=tk[2 * j + 1][:])
                nc.vector.copy_predicated(out=tk[0][:], mask=bt[1][:], data=tk[2][:])
                nc.vector.copy_predicated(out=tk[4][:], mask=bt[1][:], data=tk[6][:])
                nc.vector.copy_predicated(out=tk[0][:], mask=bt[0][:], data=tk[4][:])

                for s in range(NQ):
                    pt = ps_tr.tile([128, 128], mybir.dt.float16, name="pt",
                                    tag="pt")
                    nc.tensor.transpose(pt[:], tk[0][:, s * 128:(s + 1) * 128],
                                        ident[:])
                    nc.scalar.copy(out=wt[ot][jj0 + s][:], in_=pt[:])

        # --- GEMM ----------------------------------------------------------
        xt_r = xt_e.ap().rearrange("(kt p) t -> p kt t", p=128)
        for tb in range(NTB):
            xs = xpool.tile([128, KT, TBLK], mybir.dt.float16, name="xs",
                            tag="xs")
            nc.sync.dma_start(out=xs[:],
                              in_=xt_r[:, :, tb * TBLK:(tb + 1) * TBLK])
            for ot in range(OT):
                pm = ps_mm.tile([128, TBLK], mybir.dt.float32, name="pm",
                                tag="pm")
                for kt in range(KT):
                    nc.tensor.matmul(pm[:], lhsT=wt[ot][kt][:],
                                     rhs=xs[:, kt, :],
                                     start=(kt == 0), stop=(kt == KT - 1))
                ob = opool.tile([128, TBLK], mybir.dt.float16, name="ob",
                                tag="ob")
                nc.scalar.activation(
                    out=ob[:], in_=pm[:],
                    func=mybir.ActivationFunctionType.Identity,
                    bias=bias32[:, ot:ot + 1], scale=1.0)
                nc.sync.dma_start(
                    out=out_e[ot * 128:(ot + 1) * 128,
                              tb * TBLK:(tb + 1) * TBLK],
                    in_=ob[:])
        ctx.close()

    nc.compile()
    return nc


def _get_program():
    global _PROGRAM
    if _PROGRAM is None:
        _PROGRAM = _build_program()
    return _PROGRAM


def _shard_inputs(x, qweight, lut, bias):
    x = np.asarray(x, dtype=np.float16)
    qweight = np.asarray(qweight, dtype=np.int32)
    lut = np.asarray(lut, dtype=np.float16)
    bias = np.asarray(bias, dtype=np.float16)

    xt = x.reshape(T, IN)
    # i' = jj*128 + w  <->  i = 32w + jj ; xt_perm[i', t] = x[t, i]
    xt_perm = np.ascontiguousarray(
        xt.reshape(T, 128, 32).transpose(2, 1, 0).reshape(IN, T))

    qw_pad = np.zeros((4, O_PAD, 128), np.int32)
    qw_pad[:, :O_FULL] = qweight[:4]
    lut_pad = np.zeros((O_PAD, 16), np.float16)
    lut_pad[:O_FULL] = lut
    bias_pad = np.zeros((O_PAD, 1), np.float16)
    bias_pad[:O_FULL, 0] = bias

    in_maps = []
    for c in range(NCORES):
        sl = slice(c * O_SH, (c + 1) * O_SH)
        in_maps.append({
            "qw": np.ascontiguousarray(qw_pad[:, sl]),
            "lut": np.ascontiguousarray(lut_pad[sl]),
            "bias": np.ascontiguousarray(bias_pad[sl]),
            "xt": xt_perm,
        })
    return in_maps


def _gather(results):
    full = np.concatenate([np.asarray(r["out"]) for r in results], axis=0)
    y = full[:O_FULL].T  # [T, O_FULL]
    return np.ascontiguousarray(y.reshape(2, 2048, O_FULL), dtype=np.float16)


def kernel(x, qweight, lut, bias, w_bits=4):
    from concourse.bass_utils import run_bass_kernel_spmd

    assert int(w_bits) == 4, f"kernel hardcodes w_bits=4, got {w_bits}"
    nc = _get_program()
    in_maps = _shard_inputs(x, qweight, lut, bias)
    res = run_bass_kernel_spmd(nc, in_maps, core_ids=list(range(NCORES)))
    return _gather(res.results)


def run_timed(x, qweight, lut, bias, reps=10):
    """Run on 8 cores; return (y, per-exec wall times in ns).

    Mirrors bass2jax.run_bass_via_pjrt's multi-core path but keeps inputs
    device-resident and allocates the donated output buffers inside the jit,
    so repeated calls measure (device exec + dispatch) without host
    transfers.  NTFF profiling is unavailable under this axon build, so
    min-over-reps wall time is the hardware timing signal.
    """
    import time
    import jax
    import jax.numpy as jnp
    import numpy as np_
    from jax.sharding import Mesh, PartitionSpec
    from jax.experimental.shard_map import shard_map
    import concourse.mybir as mybir
    from concourse.bass2jax import (_bass_exec_p, install_neuronx_cc_hook,
                                    partition_id_tensor)

    install_neuronx_cc_hook()
    nc = _get_program()
    in_maps = _shard_inputs(x, qweight, lut, bias)
    n_cores = NCORES
    pid_name = nc.partition_id_tensor.name if nc.partition_id_tensor else None

    in_names, out_names, out_avals = [], [], []
    for alloc in nc.m.functions[0].allocations:
        if not isinstance(alloc, mybir.MemoryLocationSet):
            continue
        name = alloc.memorylocations[0].name
        if alloc.kind == "ExternalInput":
            if name != pid_name:
                in_names.append(name)
        elif alloc.kind == "ExternalOutput":
            out_names.append(name)
            out_avals.append(jax.core.ShapedArray(
                tuple(alloc.tensor_shape), mybir.dt.np(alloc.dtype)))
    n_params = len(in_names)

    bind_in_names = list(in_names) + list(out_names)
    if pid_name is not None:
        bind_in_names.append(pid_name)

    def _body(*args):
        operands = list(args)
        if pid_name is not None:
            operands.append(partition_id_tensor())
        outs = _bass_exec_p.bind(
            *operands,
            out_avals=tuple(out_avals),
            in_names=tuple(bind_in_names),
            out_names=tuple(out_names),
            lowering_input_output_aliases=(),
            sim_require_finite=True,
            sim_require_nnan=True,
            nc=nc,
        )
        return tuple(outs)

    n_outs = len(out_names)
    devices = jax.devices()[:n_cores]
    mesh = Mesh(np_.asarray(devices), ("core",))
    spec = PartitionSpec("core")
    sharded = jax.jit(shard_map(
        _body, mesh=mesh,
        in_specs=(spec,) * (n_params + n_outs),
        out_specs=(spec,) * n_outs,
        check_rep=False),
        donate_argnums=tuple(range(n_params, n_params + n_outs)),
        keep_unused=True)

    shardings = [jax.sharding.NamedSharding(mesh, spec)] * n_outs
    global_zero_shapes = [(n_cores * a.shape[0], *a.shape[1:]) for a in out_avals]
    make_zeros = jax.jit(
        lambda: tuple(jnp.zeros(s, a.dtype)
                      for s, a in zip(global_zero_shapes, out_avals)),
        out_shardings=tuple(shardings))

    concat_in = [
        np_.concatenate([np_.asarray(in_maps[c][nm]) for c in range(n_cores)],
                        axis=0)
        for nm in in_names
    ]
    concat_in = [jax.device_put(a, jax.sharding.NamedSharding(mesh, spec))
                 for a in concat_in]
    zeros = make_zeros()
    out_arrs = sharded(*concat_in, *zeros)
    jax.block_until_ready(out_arrs)
    times = []
    for _ in range(reps):
        zeros = make_zeros()
        jax.block_until_ready(zeros)
        t0 = time.perf_counter_ns()
        out_arrs = sharded(*concat_in, *zeros)
        jax.block_until_ready(out_arrs)
        times.append(time.perf_counter_ns() - t0)

    results = [
        {nm: np_.asarray(out_arrs[i]).reshape(n_cores, *out_avals[i].shape)[c]
         for i, nm in enumerate(out_names)}
        for c in range(n_cores)
    ]
    return _gather(results), times


# revision 21
# speedup vs baseline: 90.3251x; 90.3251x over previous
"""AnyPrecisionLinear (4-bit LUT-quantized linear) Trainium2 kernel, 8-core SPMD.

y[b,s,o] = sum_i x[b,s,i] * lut[o, code[o,i]] + bias[o]
code assembled MSB-first from bitplanes 0..3 of qweight.

Sharding (column-parallel, per hint): out_features padded 11008->11264 and
split 1408 per core; x replicated; no collectives. Output gathered on host.

Per-core pipeline (o-tiles processed in groups so the GEMM of group g
overlaps the dequant of group g+1 in the static Tile schedule):
  dequant per o-tile (128 rows):
    - bit tiles via fused bitwise shift+and tensor_scalar (int32, DVE)
    - fp16 converts of the bit tiles (DVE) for 2x-mode tree ops
    - 8 codebook "leaves" t_k = b3*d_k + lut[:,2k] on the Scalar engine
      (per-partition fp32 scale/bias APs)
    - 7-op copy_predicated binary select tree (DVE, fp16 2x mode)
    - TensorEngine 128x128 transposes into WT[i',o] (GEMM stationary layout)
  GEMM per group: for each 256-token block accumulate 32 K-tiles in PSUM;
  epilogue adds bias + casts to fp16 on the Scalar engine.

The in-feature axis is processed in a permuted order i' = jj*128 + w
(i = 32w + jj) making bit extraction and K-tiling dense; the host applies
the same permutation to x (contraction order is free).
"""

import numpy as np

IN = 4096
O_FULL = 11008
NCORES = 8
O_PAD = 11264          # 8 * 11 * 128
O_SH = O_PAD // NCORES  # 1408
OT = O_SH // 128        # 11 o-tiles
KT = IN // 128          # 32 k-tiles
T = 4096                # tokens
TBLK = 256
NTB = T // TBLK         # 16 token blocks
NQ = 8                  # bit-positions (jj) per dequant pass
NPASS = KT // NQ        # 4 passes per o-tile
GROUPS = [2, 3, 3, 3]   # o-tile pipeline groups (GEMM g overlaps dequant g+1)

# schedule/config knobs (read at build time; timeline sweeps override these)
CONFIG = {
    "mask_conv": "int16",  # None | "int16": convert b0..b2 masks (walrus
                           # requires int mask dtypes; int16 may enable 2x cp)
    "bpool_bufs": 1,
    "fpool_bufs": 2,
    "tpool_bufs": 2,
    "x_halves": False,
    "groups": GROUPS,
    "leaf_dve": 0,       # how many of the 8 leaves run on DVE tensor_scalar
    "ps_mm_bufs": 4,
    "ps_tr_bufs": 2,
}

_PROGRAM = None


def _build_program():
    import concourse.mybir as mybir
    import concourse.tile as tile
    from concourse import bacc
    from concourse.masks import make_identity
    from contextlib import ExitStack

    nc = bacc.Bacc("TRN2", target_bir_lowering=False, debug=False,
                   num_devices=NCORES)

    qw_e = nc.dram_tensor("qw", [4, O_SH, 128], mybir.dt.int32,
                          kind="ExternalInput")
    lut_e = nc.dram_tensor("lut", [O_SH, 16], mybir.dt.float16,
                           kind="ExternalInput")
    bias_e = nc.dram_tensor("bias", [O_SH, 1], mybir.dt.float16,
                            kind="ExternalInput")
    xt_e = nc.dram_tensor("xt", [IN, T], mybir.dt.float16,
                          kind="ExternalInput")
    out_e = nc.dram_tensor("out", [O_SH, T], mybir.dt.float16,
                           kind="ExternalOutput")

    with tile.TileContext(nc) as tc:
        ctx = ExitStack()
        singles = ctx.enter_context(tc.tile_pool(name="singles", bufs=1))
        qpool = ctx.enter_context(tc.tile_pool(name="qpool", bufs=1))
        bpool = ctx.enter_context(tc.tile_pool(name="bpool",
                                               bufs=CONFIG["bpool_bufs"]))
        fpool = ctx.enter_context(tc.tile_pool(name="fpool",
                                               bufs=CONFIG["fpool_bufs"]))
        tpool = ctx.enter_context(tc.tile_pool(name="tpool",
                                               bufs=CONFIG["tpool_bufs"]))
        wpool = ctx.enter_context(tc.tile_pool(name="wpool", bufs=1))
        xpool = ctx.enter_context(tc.tile_pool(
            name="xpool", bufs=3 if CONFIG["x_halves"] else 2))
        opool = ctx.enter_context(tc.tile_pool(name="opool", bufs=4))
        ps_tr = ctx.enter_context(tc.tile_pool(
            name="ps_tr", bufs=CONFIG["ps_tr_bufs"], space="PSUM"))
        ps_mm = ctx.enter_context(tc.tile_pool(
            name="ps_mm", bufs=CONFIG["ps_mm_bufs"], space="PSUM"))

        # --- constants -----------------------------------------------------
        ident = singles.tile([128, 128], mybir.dt.float16, name="ident")
        make_identity(nc, ident[:])

        lut_sb = singles.tile([128, OT, 16], mybir.dt.float16, name="lut_sb")
        nc.sync.dma_start(
            out=lut_sb[:],
            in_=lut_e.ap().rearrange("(ot p) c -> p ot c", p=128))
        lut32 = singles.tile([128, OT, 16], mybir.dt.float32, name="lut32")
        nc.vector.tensor_copy(out=lut32[:], in_=lut_sb[:])
        dq = singles.tile([128, OT, 8], mybir.dt.float32, name="dq")
        nc.vector.tensor_tensor(out=dq[:], in0=lut32[:, :, 1::2],
                                in1=lut32[:, :, 0::2],
                                op=mybir.AluOpType.subtract)

        bias_sb = singles.tile([128, OT], mybir.dt.float16, name="bias_sb")
        nc.sync.dma_start(
            out=bias_sb[:],
            in_=bias_e.ap().rearrange("(ot p) c -> p (ot c)", p=128))
        bias32 = singles.tile([128, OT], mybir.dt.float32, name="bias32")
        nc.vector.tensor_copy(out=bias32[:], in_=bias_sb[:])

        # persistent transposed weights: wt[ot][kt] is [128 i', 128 o] fp16
        wt = [[wpool.tile([128, 128], mybir.dt.float16,
                          name=f"wt_{ot}_{kt}", tag=f"wt_{ot}_{kt}")
               for kt in range(KT)] for ot in range(OT)]

        xt_r = xt_e.ap().rearrange("(kt p) t -> p kt t", p=128)

        def dequant_otile(ot):
            qt = [qpool.tile([128, 128], mybir.dt.int32, name=f"qt{p}",
                             tag=f"qt{p}") for p in range(4)]
            for p in range(4):
                nc.sync.dma_start(out=qt[p][:],
                                  in_=qw_e[p, ot * 128:(ot + 1) * 128, :])
            for ps in range(NPASS):
                jj0 = ps * NQ
                # int32 0/1 bit tiles via fused bitwise shift+and (bitwise
                # TensorScalar cannot cast, so extraction stays int32)
                bt = [bpool.tile([128, NQ * 128], mybir.dt.int32,
                                 name=f"bt{p}", tag=f"bt{p}")
                      for p in range(3)]
                for j in range(NQ):
                    for p in range(3):
                        # all-DVE: GPSIMD shares the DVE SBUF port (exclusive
                        # lock), so offloading there serializes instead
                        nc.vector.tensor_scalar(
                            out=bt[p][:, j * 128:(j + 1) * 128],
                            in0=qt[p][:],
                            scalar1=31 - (jj0 + j),
                            scalar2=1,
                            op0=mybir.AluOpType.logical_shift_right,
                            op1=mybir.AluOpType.bitwise_and,
                        )
                # b3 (LSB plane) extracted straight to exact fp16 0/1 for the
                # leaves: shift-left (bitwise, no cast), then sign-compare
                # (arith, cast allowed)
                b3i = bpool.tile([128, NQ * 128], mybir.dt.int32,
                                 name="b3i", tag="b3i")
                for j in range(NQ):
                    nc.vector.tensor_scalar(
                        out=b3i[:, j * 128:(j + 1) * 128],
                        in0=qt[3][:], scalar1=jj0 + j, scalar2=None,
                        op0=mybir.AluOpType.logical_shift_left,
                        op1=mybir.AluOpType.bypass)
                b3f = fpool.tile([128, NQ * 128], mybir.dt.float16,
                                 name="b3f", tag="b3f")
                nc.vector.tensor_scalar(
                    out=b3f[:], in0=b3i[:], scalar1=0, scalar2=None,
                    op0=mybir.AluOpType.is_lt, op1=mybir.AluOpType.bypass)
                msk = [bt[p] for p in range(3)]
                if CONFIG["mask_conv"] == "int16":
                    for p in range(3):
                        m16 = fpool.tile([128, NQ * 128], mybir.dt.int16,
                                         name=f"m16_{p}", tag=f"m16_{p}")
                        nc.vector.tensor_copy(out=m16[:], in_=bt[p][:])
                        msk[p] = m16

                tk = [tpool.tile([128, NQ * 128], mybir.dt.float16,
                                 name=f"tk{k}", tag=f"tk{k}")
                      for k in range(8)]
                for k in range(8):
                    if k < CONFIG["leaf_dve"]:
                        nc.vector.tensor_scalar(
                            out=tk[k][:], in0=b3f[:],
                            scalar1=dq[:, ot, k:k + 1],
                            scalar2=lut32[:, ot, 2 * k:2 * k + 1],
                            op0=mybir.AluOpType.mult,
                            op1=mybir.AluOpType.add,
                        )
                    else:
                        nc.scalar.activation(
                            out=tk[k][:], in_=b3f[:],
                            func=mybir.ActivationFunctionType.Identity,
                            bias=lut32[:, ot, 2 * k:2 * k + 1],
                            scale=dq[:, ot, k:k + 1],
                        )
                for j in range(4):
                    nc.vector.copy_predicated(out=tk[2 * j][:], mask=msk[2][:],
                                              data=tk[2 * j + 1][:])
                nc.vector.copy_predicated(out=tk[0][:], mask=msk[1][:],
                                          data=tk[2][:])
                nc.vector.copy_predicated(out=tk[4][:], mask=msk[1][:],
                                          data=tk[6][:])
                nc.vector.copy_predicated(out=tk[0][:], mask=msk[0][:],
                                          data=tk[4][:])

                for s in range(NQ):
                    pt = ps_tr.tile([128, 128], mybir.dt.float16, name="pt",
                                    tag="pt")
                    nc.tensor.transpose(pt[:], tk[0][:, s * 128:(s + 1) * 128],
                                        ident[:])
                    nc.scalar.copy(out=wt[ot][jj0 + s][:], in_=pt[:])

        def gemm_group(ots):
            nh = 2 if CONFIG["x_halves"] else 1
            KH = KT // nh
            for tb in range(NTB):
                tsl = slice(tb * TBLK, (tb + 1) * TBLK)
                xs = [xpool.tile([128, KH, TBLK], mybir.dt.float16,
                                 name=f"xs{h}", tag="xs") for h in range(nh)]
                for h in range(nh):
                    nc.sync.dma_start(out=xs[h][:],
                                      in_=xt_r[:, h * KH:(h + 1) * KH, tsl])
                for ot in ots:
                    pm = ps_mm.tile([128, TBLK], mybir.dt.float32, name="pm",
                                    tag="pm")
                    for kt in range(KT):
                        nc.tensor.matmul(pm[:], lhsT=wt[ot][kt][:],
                                         rhs=xs[kt // KH][:, kt % KH, :],
                                         start=(kt == 0), stop=(kt == KT - 1))
                    ob = opool.tile([128, TBLK], mybir.dt.float16, name="ob",
                                    tag="ob")
                    nc.scalar.activation(
                        out=ob[:], in_=pm[:],
                        func=mybir.ActivationFunctionType.Identity,
                        bias=bias32[:, ot:ot + 1], scale=1.0)
                    nc.sync.dma_start(
                        out=out_e[ot * 128:(ot + 1) * 128,
                                  tb * TBLK:(tb + 1) * TBLK],
                        in_=ob[:])

        # --- grouped pipeline: dequant(g0), then for each g: dequant(g+1)
        # interleaves (by engine independence) with gemm(g) ---------------
        groups = []
        o0 = 0
        for sz in CONFIG["groups"]:
            groups.append(list(range(o0, o0 + sz)))
            o0 += sz
        def body():
            for g, ots in enumerate(groups):
                for ot in ots:
                    dequant_otile(ot)
                if g > 0:
                    gemm_group(groups[g - 1])
            gemm_group(groups[-1])

        if CONFIG.get("loop_n"):
            # timing variant: run the whole pipeline loop_n times inside the
            # NEFF (back-edge is a full barrier, so iterations are idempotent)
            with tc.For_i(0, CONFIG["loop_n"], 1):
                body()
        else:
            body()
        ctx.close()

    nc.compile()
    return nc


def _get_program():
    global _PROGRAM
    if _PROGRAM is None:
        _PROGRAM = _build_program()
    return _PROGRAM


def _shard_inputs(x, qweight, lut, bias):
    x = np.asarray(x, dtype=np.float16)
    qweight = np.asarray(qweight, dtype=np.int32)
    lut = np.asarray(lut, dtype=np.float16)
    bias = np.asarray(bias, dtype=np.float16)

    xt = x.reshape(T, IN)
    # i' = jj*128 + w  <->  i = 32w + jj ; xt_perm[i', t] = x[t, i]
    xt_perm = np.ascontiguousarray(
        xt.reshape(T, 128, 32).transpose(2, 1, 0).reshape(IN, T))

    qw_pad = np.zeros((4, O_PAD, 128), np.int32)
    qw_pad[:, :O_FULL] = qweight[:4]
    lut_pad = np.zeros((O_PAD, 16), np.float16)
    lut_pad[:O_FULL] = lut
    bias_pad = np.zeros((O_PAD, 1), np.float16)
    bias_pad[:O_FULL, 0] = bias

    in_maps = []
    for c in range(NCORES):
        sl = slice(c * O_SH, (c + 1) * O_SH)
        in_maps.append({
            "qw": np.ascontiguousarray(qw_pad[:, sl]),
            "lut": np.ascontiguousarray(lut_pad[sl]),
            "bias": np.ascontiguousarray(bias_pad[sl]),
            "xt": xt_perm,
        })
    return in_maps


def _gather(results):
    full = np.concatenate([np.asarray(r["out"]) for r in results], axis=0)
    y = full[:O_FULL].T  # [T, O_FULL]
    return np.ascontiguousarray(y.reshape(2, 2048, O_FULL), dtype=np.float16)


def kernel(x, qweight, lut, bias, w_bits=4):
    from concourse.bass_utils import run_bass_kernel_spmd

    assert int(w_bits) == 4, f"kernel hardcodes w_bits=4, got {w_bits}"
    nc = _get_program()
    in_maps = _shard_inputs(x, qweight, lut, bias)
    res = run_bass_kernel_spmd(nc, in_maps, core_ids=list(range(NCORES)))
    return _gather(res.results)


def _time_nc(nc, in_maps, reps=5):
    """Min wall-clock (ns) of dispatching one NEFF exec of `nc` on 8 cores,
    inputs device-resident, donated zero output buffers made per rep."""
    import time
    import jax
    import jax.numpy as jnp
    from jax.sharding import Mesh, PartitionSpec, NamedSharding
    from jax.experimental.shard_map import shard_map
    import concourse.mybir as mybir
    from concourse.bass2jax import (_bass_exec_p, install_neuronx_cc_hook,
                                    partition_id_tensor)

    install_neuronx_cc_hook()
    n_cores = NCORES
    pid_name = nc.partition_id_tensor.name if nc.partition_id_tensor else None
    in_names, out_names, out_avals = [], [], []
    for alloc in nc.m.functions[0].allocations:
        if not isinstance(alloc, mybir.MemoryLocationSet):
            continue
        name = alloc.memorylocations[0].name
        if alloc.kind == "ExternalInput":
            if name != pid_name:
                in_names.append(name)
        elif alloc.kind == "ExternalOutput":
            out_names.append(name)
            out_avals.append(jax.core.ShapedArray(
                tuple(alloc.tensor_shape), mybir.dt.np(alloc.dtype)))
    n_params = len(in_names)
    n_outs = len(out_names)
    bind_in_names = list(in_names) + list(out_names)
    if pid_name is not None:
        bind_in_names.append(pid_name)

    def _body(*args):
        operands = list(args)
        if pid_name is not None:
            operands.append(partition_id_tensor())
        return tuple(_bass_exec_p.bind(
            *operands,
            out_avals=tuple(out_avals),
            in_names=tuple(bind_in_names),
            out_names=tuple(out_names),
            lowering_input_output_aliases=(),
            sim_require_finite=True,
            sim_require_nnan=True,
            nc=nc,
        ))

    devices = jax.devices()[:n_cores]
    mesh = Mesh(np.asarray(devices), ("core",))
    spec = PartitionSpec("core")
    sh = NamedSharding(mesh, spec)
    sharded = jax.jit(shard_map(
        _body, mesh=mesh,
        in_specs=(spec,) * (n_params + n_outs),
        out_specs=(spec,) * n_outs,
        check_rep=False),
        donate_argnums=tuple(range(n_params, n_params + n_outs)),
        keep_unused=True)
    gz = [(n_cores * a.shape[0], *a.shape[1:]) for a in out_avals]
    make_zeros = jax.jit(
        lambda: tuple(jnp.zeros(s_, a.dtype) for s_, a in zip(gz, out_avals)),
        out_shardings=tuple([sh] * n_outs))
    concat_in = [jax.device_put(
        np.concatenate([np.asarray(in_maps[c][nm]) for c in range(n_cores)],
                       axis=0), sh) for nm in in_names]
    out_arrs = sharded(*concat_in, *make_zeros())
    jax.block_until_ready(out_arrs)
    walls = []
    for _ in range(reps):
        z = make_zeros()
        jax.block_until_ready(z)
        t0 = time.perf_counter_ns()
        out_arrs = sharded(*concat_in, *z)
        jax.block_until_ready(out_arrs)
        walls.append(time.perf_counter_ns() - t0)
    results = [
        {nm: np.asarray(out_arrs[i]).reshape(n_cores, *out_avals[i].shape)[c]
         for i, nm in enumerate(out_names)}
        for c in range(n_cores)
    ]
    return walls, results


def run_timed(x, qweight, lut, bias, reps=5, loop_n=16):
    """Return (y, walls_1, walls_K, per_exec_ns).

    Axon dispatch costs ~80ms/call, so device time is measured
    differentially: program B runs the identical pipeline loop_n times
    inside the NEFF (tc.For_i); per-exec = (minB - minA)/(loop_n - 1).
    """
    global _PROGRAM
    in_maps = _shard_inputs(x, qweight, lut, bias)

    CONFIG["loop_n"] = None
    _PROGRAM = None
    ncA = _get_program()
    walls1, results = _time_nc(ncA, in_maps, reps=reps)

    CONFIG["loop_n"] = loop_n
    _PROGRAM = None
    ncB = _get_program()
    wallsK, _ = _time_nc(ncB, in_maps, reps=reps)
    CONFIG["loop_n"] = None
    _PROGRAM = None

    per_exec = (min(wallsK) - min(walls1)) / (loop_n - 1)
    return _gather(results), walls1, wallsK, per_exec


def np_arr(x):
    return np.asarray(x)


# revision 23
# speedup vs baseline: 387.3790x; 4.2887x over previous
"""AnyPrecisionLinear (4-bit LUT-quantized linear) Trainium2 kernel, 8-core SPMD.

y[b,s,o] = sum_i x[b,s,i] * lut[o, code[o,i]] + bias[o]
code assembled MSB-first from bitplanes 0..3 of qweight.

Sharding (column-parallel, per hint): out_features padded 11008->11264 and
split 1408 per core; x replicated; no collectives. Output gathered on host.

Per-core pipeline (o-tiles processed in groups so the GEMM of group g
overlaps the dequant of group g+1 in the static Tile schedule):
  dequant per o-tile (128 rows):
    - bit tiles via fused bitwise shift+and tensor_scalar (int32, DVE)
    - fp16 converts of the bit tiles (DVE) for 2x-mode tree ops
    - 8 codebook "leaves" t_k = b3*d_k + lut[:,2k] on the Scalar engine
      (per-partition fp32 scale/bias APs)
    - 7-op copy_predicated binary select tree (DVE, fp16 2x mode)
    - TensorEngine 128x128 transposes into WT[i',o] (GEMM stationary layout)
  GEMM per group: for each 256-token block accumulate 32 K-tiles in PSUM;
  epilogue adds bias + casts to fp16 on the Scalar engine.

The in-feature axis is processed in a permuted order i' = jj*128 + w
(i = 32w + jj) making bit extraction and K-tiling dense; the host applies
the same permutation to x (contraction order is free).
"""

import numpy as np

IN = 4096
O_FULL = 11008
NCORES = 8
O_PAD = 11264          # 8 * 11 * 128
O_SH = O_PAD // NCORES  # 1408
OT = O_SH // 128        # 11 o-tiles
KT = IN // 128          # 32 k-tiles
T = 4096                # tokens
TBLK = 256
NTB = T // TBLK         # 16 token blocks
NQ = 8                  # bit-positions (jj) per dequant pass
NPASS = KT // NQ        # 4 passes per o-tile
GROUPS = [2, 3, 3, 3]   # o-tile pipeline groups (GEMM g overlaps dequant g+1)

# schedule/config knobs (read at build time; timeline sweeps override these)
CONFIG = {
    "mask_conv": "int16",  # None | "int16": convert b0..b2 masks (walrus
                           # requires int mask dtypes; int16 may enable 2x cp)
    "bpool_bufs": 1,
    "fpool_bufs": 2,
    "tpool_bufs": 2,
    "x_halves": False,
    "groups": GROUPS,
    "leaf_dve": 0,       # how many of the 8 leaves run on DVE tensor_scalar
    "ps_mm_bufs": 4,
    "ps_tr_bufs": 2,
    "loop_n": None,
    # timing-only ablations (wrong results, same structure):
    "skip_tree": False,
    "skip_gemm": False,
    "skip_dequant": False,
}

_PROGRAM = None


def _build_program():
    import concourse.mybir as mybir
    import concourse.tile as tile
    from concourse import bacc
    from concourse.masks import make_identity
    from contextlib import ExitStack

    nc = bacc.Bacc("TRN2", target_bir_lowering=False, debug=False,
                   num_devices=NCORES)

    qw_e = nc.dram_tensor("qw", [4, O_SH, 128], mybir.dt.int32,
                          kind="ExternalInput")
    lut_e = nc.dram_tensor("lut", [O_SH, 16], mybir.dt.float16,
                           kind="ExternalInput")
    bias_e = nc.dram_tensor("bias", [O_SH, 1], mybir.dt.float16,
                            kind="ExternalInput")
    # x pre-tiled on host to [tb, p, kt, u] so each token-block slab DMA
    # reads 16KB contiguous per partition (vs 512B strided chunks)
    xt_e = nc.dram_tensor("xt", [NTB, 128, KT, TBLK], mybir.dt.float16,
                          kind="ExternalInput")
    out_e = nc.dram_tensor("out", [O_SH, T], mybir.dt.float16,
                           kind="ExternalOutput")

    with tile.TileContext(nc) as tc:
        ctx = ExitStack()
        singles = ctx.enter_context(tc.tile_pool(name="singles", bufs=1))
        qpool = ctx.enter_context(tc.tile_pool(name="qpool", bufs=1))
        bpool = ctx.enter_context(tc.tile_pool(name="bpool",
                                               bufs=CONFIG["bpool_bufs"]))
        fpool = ctx.enter_context(tc.tile_pool(name="fpool",
                                               bufs=CONFIG["fpool_bufs"]))
        tpool = ctx.enter_context(tc.tile_pool(name="tpool",
                                               bufs=CONFIG["tpool_bufs"]))
        wpool = ctx.enter_context(tc.tile_pool(name="wpool", bufs=1))
        xpool = ctx.enter_context(tc.tile_pool(
            name="xpool", bufs=3 if CONFIG["x_halves"] else 2))
        opool = ctx.enter_context(tc.tile_pool(name="opool", bufs=4))
        ps_tr = ctx.enter_context(tc.tile_pool(
            name="ps_tr", bufs=CONFIG["ps_tr_bufs"], space="PSUM"))
        ps_mm = ctx.enter_context(tc.tile_pool(
            name="ps_mm", bufs=CONFIG["ps_mm_bufs"], space="PSUM"))

        # --- constants -----------------------------------------------------
        ident = singles.tile([128, 128], mybir.dt.float16, name="ident")
        make_identity(nc, ident[:])

        lut_sb = singles.tile([128, OT, 16], mybir.dt.float16, name="lut_sb")
        nc.sync.dma_start(
            out=lut_sb[:],
            in_=lut_e.ap().rearrange("(ot p) c -> p ot c", p=128))
        lut32 = singles.tile([128, OT, 16], mybir.dt.float32, name="lut32")
        nc.vector.tensor_copy(out=lut32[:], in_=lut_sb[:])
        dq = singles.tile([128, OT, 8], mybir.dt.float32, name="dq")
        nc.vector.tensor_tensor(out=dq[:], in0=lut32[:, :, 1::2],
                                in1=lut32[:, :, 0::2],
                                op=mybir.AluOpType.subtract)

        bias_sb = singles.tile([128, OT], mybir.dt.float16, name="bias_sb")
        nc.sync.dma_start(
            out=bias_sb[:],
            in_=bias_e.ap().rearrange("(ot p) c -> p (ot c)", p=128))
        bias32 = singles.tile([128, OT], mybir.dt.float32, name="bias32")
        nc.vector.tensor_copy(out=bias32[:], in_=bias_sb[:])

        # persistent transposed weights: wt[ot][kt] is [128 i', 128 o] fp16
        wt = [[wpool.tile([128, 128], mybir.dt.float16,
                          name=f"wt_{ot}_{kt}", tag=f"wt_{ot}_{kt}")
               for kt in range(KT)] for ot in range(OT)]
        if CONFIG["skip_dequant"]:
            for ot in range(OT):
                for kt in range(KT):
                    nc.vector.memset(wt[ot][kt][:], 0.0)


        def dequant_otile(ot):
            if CONFIG["skip_dequant"]:
                return
            qt = [qpool.tile([128, 128], mybir.dt.int32, name=f"qt{p}",
                             tag=f"qt{p}") for p in range(4)]
            for p in range(4):
                nc.sync.dma_start(out=qt[p][:],
                                  in_=qw_e[p, ot * 128:(ot + 1) * 128, :])
            for ps in range(NPASS):
                jj0 = ps * NQ
                # int32 0/1 bit tiles via fused bitwise shift+and (bitwise
                # TensorScalar cannot cast, so extraction stays int32)
                bt = [bpool.tile([128, NQ * 128], mybir.dt.int32,
                                 name=f"bt{p}", tag=f"bt{p}")
                      for p in range(3)]
                for j in range(NQ):
                    for p in range(3):
                        # all-DVE: GPSIMD shares the DVE SBUF port (exclusive
                        # lock), so offloading there serializes instead
                        nc.vector.tensor_scalar(
                            out=bt[p][:, j * 128:(j + 1) * 128],
                            in0=qt[p][:],
                            scalar1=31 - (jj0 + j),
                            scalar2=1,
                            op0=mybir.AluOpType.logical_shift_right,
                            op1=mybir.AluOpType.bitwise_and,
                        )
                # b3 (LSB plane) extracted straight to exact fp16 0/1 for the
                # leaves: shift-left (bitwise, no cast), then sign-compare
                # (arith, cast allowed)
                b3i = bpool.tile([128, NQ * 128], mybir.dt.int32,
                                 name="b3i", tag="b3i")
                for j in range(NQ):
                    nc.vector.tensor_scalar(
                        out=b3i[:, j * 128:(j + 1) * 128],
                        in0=qt[3][:], scalar1=jj0 + j, scalar2=None,
                        op0=mybir.AluOpType.logical_shift_left,
                        op1=mybir.AluOpType.bypass)
                b3f = fpool.tile([128, NQ * 128], mybir.dt.float16,
                                 name="b3f", tag="b3f")
                nc.vector.tensor_scalar(
                    out=b3f[:], in0=b3i[:], scalar1=0, scalar2=None,
                    op0=mybir.AluOpType.is_lt, op1=mybir.AluOpType.bypass)
                msk = [bt[p] for p in range(3)]
                if CONFIG["mask_conv"] == "int16":
                    for p in range(3):
                        m16 = fpool.tile([128, NQ * 128], mybir.dt.int16,
                                         name=f"m16_{p}", tag=f"m16_{p}")
                        nc.vector.tensor_copy(out=m16[:], in_=bt[p][:])
                        msk[p] = m16

                tk = [tpool.tile([128, NQ * 128], mybir.dt.float16,
                                 name=f"tk{k}", tag=f"tk{k}")
                      for k in range(8)]
                for k in range(8):
                    if k < CONFIG["leaf_dve"]:
                        nc.vector.tensor_scalar(
                            out=tk[k][:], in0=b3f[:],
                            scalar1=dq[:, ot, k:k + 1],
                            scalar2=lut32[:, ot, 2 * k:2 * k + 1],
                            op0=mybir.AluOpType.mult,
                            op1=mybir.AluOpType.add,
                        )
                    else:
                        nc.scalar.activation(
                            out=tk[k][:], in_=b3f[:],
                            func=mybir.ActivationFunctionType.Identity,
                            bias=lut32[:, ot, 2 * k:2 * k + 1],
                            scale=dq[:, ot, k:k + 1],
                        )
                if not CONFIG["skip_tree"]:
                    for j in range(4):
                        nc.vector.copy_predicated(out=tk[2 * j][:],
                                                  mask=msk[2][:],
                                                  data=tk[2 * j + 1][:])
                    nc.vector.copy_predicated(out=tk[0][:], mask=msk[1][:],
                                              data=tk[2][:])
                    nc.vector.copy_predicated(out=tk[4][:], mask=msk[1][:],
                                              data=tk[6][:])
                    nc.vector.copy_predicated(out=tk[0][:], mask=msk[0][:],
                                              data=tk[4][:])

                for s in range(NQ):
                    pt = ps_tr.tile([128, 128], mybir.dt.float16, name="pt",
                                    tag="pt")
                    nc.tensor.transpose(pt[:], tk[0][:, s * 128:(s + 1) * 128],
                                        ident[:])
                    nc.scalar.copy(out=wt[ot][jj0 + s][:], in_=pt[:])

        def gemm_group(ots):
            if CONFIG["skip_gemm"]:
                return
            nh = 2 if CONFIG["x_halves"] else 1
            KH = KT // nh
            for tb in range(NTB):
                xs = [xpool.tile([128, KH, TBLK], mybir.dt.float16,
                                 name=f"xs{h}", tag="xs") for h in range(nh)]
                for h in range(nh):
                    nc.sync.dma_start(
                        out=xs[h][:],
                        in_=xt_e[tb, :, h * KH:(h + 1) * KH, :])
                for ot in ots:
                    pm = ps_mm.tile([128, TBLK], mybir.dt.float32, name="pm",
                                    tag="pm")
                    for kt in range(KT):
                        nc.tensor.matmul(pm[:], lhsT=wt[ot][kt][:],
                                         rhs=xs[kt // KH][:, kt % KH, :],
                                         start=(kt == 0), stop=(kt == KT - 1))
                    ob = opool.tile([128, TBLK], mybir.dt.float16, name="ob",
                                    tag="ob")
                    nc.scalar.activation(
                        out=ob[:], in_=pm[:],
                        func=mybir.ActivationFunctionType.Identity,
                        bias=bias32[:, ot:ot + 1], scale=1.0)
                    nc.sync.dma_start(
                        out=out_e[ot * 128:(ot + 1) * 128,
                                  tb * TBLK:(tb + 1) * TBLK],
                        in_=ob[:])

        # --- grouped pipeline: dequant(g0), then for each g: dequant(g+1)
        # interleaves (by engine independence) with gemm(g) ---------------
        groups = []
        o0 = 0
        for sz in CONFIG["groups"]:
            groups.append(list(range(o0, o0 + sz)))
            o0 += sz
        def body():
            for g, ots in enumerate(groups):
                for ot in ots:
                    dequant_otile(ot)
                if g > 0:
                    gemm_group(groups[g - 1])
            gemm_group(groups[-1])

        if CONFIG.get("loop_n"):
            # timing variant: run the whole pipeline loop_n times inside the
            # NEFF (back-edge is a full barrier, so iterations are idempotent)
            with tc.For_i(0, CONFIG["loop_n"], 1):
                body()
        else:
            body()
        ctx.close()

    nc.compile()
    return nc


def _get_program():
    global _PROGRAM
    if _PROGRAM is None:
        _PROGRAM = _build_program()
    return _PROGRAM


def _shard_inputs(x, qweight, lut, bias):
    x = np.asarray(x, dtype=np.float16)
    qweight = np.asarray(qweight, dtype=np.int32)
    lut = np.asarray(lut, dtype=np.float16)
    bias = np.asarray(bias, dtype=np.float16)

    xt = x.reshape(T, IN)
    # i' = jj*128 + w  <->  i = 32w + jj ; xt_perm[i', t] = x[t, i]
    xt_perm = xt.reshape(T, 128, 32).transpose(2, 1, 0).reshape(IN, T)
    # re-tile to [tb, p, kt, u]: per-partition-contiguous slab DMAs
    xt_perm = np.ascontiguousarray(
        xt_perm.reshape(KT, 128, NTB, TBLK).transpose(2, 1, 0, 3))

    qw_pad = np.zeros((4, O_PAD, 128), np.int32)
    qw_pad[:, :O_FULL] = qweight[:4]
    lut_pad = np.zeros((O_PAD, 16), np.float16)
    lut_pad[:O_FULL] = lut
    bias_pad = np.zeros((O_PAD, 1), np.float16)
    bias_pad[:O_FULL, 0] = bias

    in_maps = []
    for c in range(NCORES):
        sl = slice(c * O_SH, (c + 1) * O_SH)
        in_maps.append({
            "qw": np.ascontiguousarray(qw_pad[:, sl]),
            "lut": np.ascontiguousarray(lut_pad[sl]),
            "bias": np.ascontiguousarray(bias_pad[sl]),
            "xt": xt_perm,
        })
    return in_maps


def _gather(results):
    full = np.concatenate([np.asarray(r["out"]) for r in results], axis=0)
    y = full[:O_FULL].T  # [T, O_FULL]
    return np.ascontiguousarray(y.reshape(2, 2048, O_FULL), dtype=np.float16)


def kernel(x, qweight, lut, bias, w_bits=4):
    from concourse.bass_utils import run_bass_kernel_spmd

    assert int(w_bits) == 4, f"kernel hardcodes w_bits=4, got {w_bits}"
    nc = _get_program()
    in_maps = _shard_inputs(x, qweight, lut, bias)
    res = run_bass_kernel_spmd(nc, in_maps, core_ids=list(range(NCORES)))
    return _gather(res.results)


def _time_nc(nc, in_maps, reps=5):
    """Min wall-clock (ns) of dispatching one NEFF exec of `nc` on 8 cores,
    inputs device-resident, donated zero output buffers made per rep."""
    import time
    import jax
    import jax.numpy as jnp
    from jax.sharding import Mesh, PartitionSpec, NamedSharding
    from jax.experimental.shard_map import shard_map
    import concourse.mybir as mybir
    from concourse.bass2jax import (_bass_exec_p, install_neuronx_cc_hook,
                                    partition_id_tensor)

    install_neuronx_cc_hook()
    n_cores = NCORES
    pid_name = nc.partition_id_tensor.name if nc.partition_id_tensor else None
    in_names, out_names, out_avals = [], [], []
    for alloc in nc.m.functions[0].allocations:
        if not isinstance(alloc, mybir.MemoryLocationSet):
            continue
        name = alloc.memorylocations[0].name
        if alloc.kind == "ExternalInput":
            if name != pid_name:
                in_names.append(name)
        elif alloc.kind == "ExternalOutput":
            out_names.append(name)
            out_avals.append(jax.core.ShapedArray(
                tuple(alloc.tensor_shape), mybir.dt.np(alloc.dtype)))
    n_params = len(in_names)
    n_outs = len(out_names)
    bind_in_names = list(in_names) + list(out_names)
    if pid_name is not None:
        bind_in_names.append(pid_name)

    def _body(*args):
        operands = list(args)
        if pid_name is not None:
            operands.append(partition_id_tensor())
        return tuple(_bass_exec_p.bind(
            *operands,
            out_avals=tuple(out_avals),
            in_names=tuple(bind_in_names),
            out_names=tuple(out_names),
            lowering_input_output_aliases=(),
            sim_require_finite=True,
            sim_require_nnan=True,
            nc=nc,
        ))

    devices = jax.devices()[:n_cores]
    mesh = Mesh(np.asarray(devices), ("core",))
    spec = PartitionSpec("core")
    sh = NamedSharding(mesh, spec)
    sharded = jax.jit(shard_map(
        _body, mesh=mesh,
        in_specs=(spec,) * (n_params + n_outs),
        out_specs=(spec,) * n_outs,
        check_rep=False),
        donate_argnums=tuple(range(n_params, n_params + n_outs)),
        keep_unused=True)
    gz = [(n_cores * a.shape[0], *a.shape[1:]) for a in out_avals]
    make_zeros = jax.jit(
        lambda: tuple(jnp.zeros(s_, a.dtype) for s_, a in zip(gz, out_avals)),
        out_shardings=tuple([sh] * n_outs))
    concat_in = [jax.device_put(
        np.concatenate([np.asarray(in_maps[c][nm]) for c in range(n_cores)],
                       axis=0), sh) for nm in in_names]
    out_arrs = sharded(*concat_in, *make_zeros())
    jax.block_until_ready(out_arrs)
    walls = []
    for _ in range(reps):
        z = make_zeros()
        jax.block_until_ready(z)
        t0 = time.perf_counter_ns()
        out_arrs = sharded(*concat_in, *z)
        jax.block_until_ready(out_arrs)
        walls.append(time.perf_counter_ns() - t0)
    results = [
        {nm: np.asarray(out_arrs[i]).reshape(n_cores, *out_avals[i].shape)[c]
         for i, nm in enumerate(out_names)}
        for c in range(n_cores)
    ]
    return walls, results


def run_timed(x, qweight, lut, bias, reps=5, loop_n=16):
    """Return (y, walls_1, walls_K, per_exec_ns).

    Axon dispatch costs ~80ms/call, so device time is measured
    differentially: program B runs the identical pipeline loop_n times
    inside the NEFF (tc.For_i); per-exec = (minB - minA)/(loop_n - 1).
    """
    global _PROGRAM
    in_maps = _shard_inputs(x, qweight, lut, bias)

    CONFIG["loop_n"] = None
    _PROGRAM = None
    ncA = _get_program()
    walls1, results = _time_nc(ncA, in_maps, reps=reps)

    CONFIG["loop_n"] = loop_n
    _PROGRAM = None
    ncB = _get_program()
    wallsK, _ = _time_nc(ncB, in_maps, reps=reps)
    CONFIG["loop_n"] = None
    _PROGRAM = None

    per_exec = (min(wallsK) - min(walls1)) / (loop_n - 1)
    return _gather(results), walls1, wallsK, per_exec


def np_arr(x):
    return np.asarray(x)
